# revision 19
# baseline (speedup 1.0000x reference)
"""Trainium2 Bass kernel for the 8-model batch-functional CNN.

Sharding: one hypernetwork model per NeuronCore (8 models / 8 cores).
Each core runs the full 7-conv + 2-fc stack for its model over all 128
images, activations resident in SBUF.

Layout: SBUF activations are [128 partitions = 4 image-groups x 32
channels, imgs, H+2, W+2] (zero-padded borders).  3x3 convs are 9
PSUM-accumulated fp32r matmuls with block-diagonal [128,128] weights
and dy/dx-shifted rhs views; matmuls are grouped taps-outer over up to
8 PSUM banks so walrus's weight-tile cache (ldw-opt) elides redundant
LDWEIGHTS.  conv0 uses a host-side im2col (K=108).  MaxPool2d(2) is
two strided tensor_max ops split over DVE and Pool.  All conv matmuls
stream F=512 (PSUM-bank-sized) moving tiles.  fc7 runs X7-stationary
on the diagonal 32x32 PE quadrants, fc8 after a PE transpose (bias
applied post-transpose so it is a per-partition Act bias).

Startup is DMA-latency critical: chunk-0 images load per-image (4KB
per-partition packets) spread over the Pool/SP/Act queues with the
conv1 weights on their own queue so the PE starts ~12us in; steady
chunks load as one 16KB-packet DMA issued a full chunk ahead.  conv0
for chunk ch+1 is issued between conv2(ch) and conv3(ch) so its
activations land before the PE reaches conv1(ch+1).
"""
import sys

sys.path.insert(0, "/opt/trn_rl_repo")
import numpy as np

N_MODELS = 8
N_IMG = 128
IMG_PER_GROUP = 32   # images assigned to each of the 4 partition groups
CHUNK = 4            # images per group per chunk through conv0..conv4
N_CHUNKS = IMG_PER_GROUP // CHUNK


def round_fp32r(a):
    a = np.ascontiguousarray(a, dtype=np.float32)
    b = a.view(np.uint32)
    low = b & np.uint32(0xFFF)
    bit12 = (b >> np.uint32(12)) & np.uint32(1)
    up = (low > 0x800) | ((low == 0x800) & (bit12 == 1))
    out = (b & np.uint32(0xFFFFF000)) + (up.astype(np.uint32) << np.uint32(12))
    return out.view(np.float32)


def _enable_ldw_opt():
    """Turn on walrus's weight-tile cache so back-to-back matmuls with the
    same stationary operand skip the redundant LDWEIGHTS."""
    import concourse.bass_utils as bu
    if getattr(bu.run_command, "_ldw_patched", False):
        return
    orig = bu.run_command

    def run_command(cmd, *a, **kw):
        cmd = ["--enable-ldw-opt=true" if c == "--enable-ldw-opt=false" else c
               for c in cmd]
        return orig(cmd, *a, **kw)

    run_command._ldw_patched = True
    bu.run_command = run_command


def _build_program():
    import concourse.bacc as bacc
    import concourse.tile as tile
    from concourse import mybir

    _enable_ldw_opt()

    f32 = mybir.dt.float32
    f32r = mybir.dt.float32r
    Relu = mybir.ActivationFunctionType.Relu
    Ident = mybir.ActivationFunctionType.Identity
    Copy = mybir.ActivationFunctionType.Copy

    nc = bacc.Bacc("TRN2", target_bir_lowering=False, debug=False)

    x0s_d = nc.declare_dram_parameter("x0s", [108, IMG_PER_GROUP, 32, 32], f32r, isOutput=False)
    lt0b_d = nc.declare_dram_parameter("lt0b", [128, 138], f32r, isOutput=False)
    lt16_d = nc.declare_dram_parameter("lt16", [128, 6, 9, 128], f32r, isOutput=False)
    lt7_d = nc.declare_dram_parameter("lt7", [128, 16, 256], f32r, isOutput=False)
    smalls_d = nc.declare_dram_parameter("smalls", [128, 150], f32r, isOutput=False)
    out_d = nc.declare_dram_parameter("out", [10, N_IMG], f32, isOutput=True)

    conv_h = {1: 32, 2: 32, 3: 16, 4: 16, 5: 8, 6: 8}
    pools_after = {2, 4, 6}
    tile_imgs = {1: 1, 2: 1, 3: 1, 4: 1, 5: 4, 6: 4}
    TAPS = [(dy, dx) for dy in (-1, 0, 1) for dx in (-1, 0, 1)]

    with tile.TileContext(nc) as tc:
        with tc.tile_pool(name="wpool", bufs=1) as wpool, \
             tc.tile_pool(name="acts", bufs=1) as acts, \
             tc.tile_pool(name="x0pool", bufs=2) as x0pool, \
             tc.tile_pool(name="tmp", bufs=2) as tmp, \
             tc.tile_pool(name="persist", bufs=1) as persist, \
             tc.tile_pool(name="cps", bufs=8, space="PSUM") as cps:

            lt0b = wpool.tile([128, 138], f32r, tag="lt0b")
            lt16 = wpool.tile([128, 6, 9, 128], f32r, tag="lt16")
            lt7 = wpool.tile([128, 16, 256], f32r, tag="lt7")
            smalls = wpool.tile([128, 150], f32r, tag="smalls")

            lt0 = lt0b[0:108, 0:128]
            bias = lt0b[:, 128:138].bitcast(f32)     # [128, 10] conv/fc biases
            idt = smalls[:, 0:128]                   # f32r identity
            b7cols = smalls[:, 128:130].bitcast(f32)  # [128, 2] fc7 bias halves
            lt8 = smalls[:, 130:150].rearrange("p (h o) -> p h o", h=2)

            # persistent buffers (across chunks)
            x45 = persist.tile([128, IMG_PER_GROUP, 10, 10], f32r, tag="x45")
            x56 = persist.tile([128, IMG_PER_GROUP, 10, 10], f32r, tag="x56")
            x7 = persist.tile([128, IMG_PER_GROUP, 6, 6], f32r, tag="x7")

            # conv0..conv4 SBUF activation buffers (borders zeroed once,
            # interiors fully rewritten every chunk)
            x1 = acts.tile([128, CHUNK, 34, 34], f32r, tag="big")
            x1b = acts.tile([128, CHUNK, 34, 34], f32r, tag="big2")
            xm = acts.tile([128, CHUNK, 18, 18], f32r, tag="med")
            xm2 = acts.tile([128, CHUNK, 18, 18], f32r, tag="med2")

            # ---- startup DMAs, ordered per queue for earliest PE start ----
            x0tiles = [None] * N_CHUNKS
            x0tiles[0] = x0pool.tile([108, CHUNK, 32, 32], f32r, tag="x0c",
                                     name="x0c")
            x0c0 = x0tiles[0]
            # Pool queue: img0, then conv1/conv3/conv4 weights
            nc.gpsimd.dma_start(out=x0c0[:, 0], in_=x0s_d[:, 0])
            nc.gpsimd.dma_start(out=lt16[:, 0], in_=lt16_d[:, 0])
            nc.gpsimd.dma_start(out=lt16[:, 2], in_=lt16_d[:, 2])
            nc.gpsimd.dma_start(out=lt16[:, 4], in_=lt16_d[:, 4])
            # SP queue: conv0 weights+biases, img1, small tensors, conv2 w
            nc.sync.dma_start(out=lt0b[:], in_=lt0b_d[:])
            nc.sync.dma_start(out=x0c0[:, 1], in_=x0s_d[:, 1])
            nc.sync.dma_start(out=smalls[:], in_=smalls_d[:])
            nc.sync.dma_start(out=lt16[:, 1], in_=lt16_d[:, 1])
            nc.sync.dma_start(out=lt16[:, 5], in_=lt16_d[:, 5])
            # Act queue (ACT_TABLE_LOAD runs first): just img2 and img3 so
            # the last image's x1 write beats the PE to conv1
            nc.scalar.dma_start(out=x0c0[:, 2], in_=x0s_d[:, 2])
            nc.scalar.dma_start(out=x0c0[:, 3], in_=x0s_d[:, 3])
            nc.scalar.dma_start(out=lt16[:, 3], in_=lt16_d[:, 3])

            # ---- one-time border zeroing (alternate DVE / Pool) ----
            k_ms = 0

            def memset0(ap):
                nonlocal k_ms
                eng = nc.vector if k_ms % 2 == 0 else nc.gpsimd
                eng.memset(ap.bitcast(f32), 0.0)
                k_ms += 1

            for buf, n in ((x1, 34), (x1b, 34), (xm, 18), (xm2, 18)):
                memset0(buf[:, :, 0, :])
                memset0(buf[:, :, n - 1, :])
                memset0(buf[:, :, 1:n - 1, 0])
                memset0(buf[:, :, 1:n - 1, n - 1])
            for buf, n in ((x45, 10), (x56, 10), (x7, 6)):
                memset0(buf[:, :, 0, :])
                memset0(buf[:, :, n - 1, :])
                memset0(buf[:, :, 1:n - 1, 0])
                memset0(buf[:, :, 1:n - 1, n - 1])

            def conv_layer(L, xk, xn, tiles, ti, h, rows, glob_dst,
                           gsz=8):
                """One 3x3 conv (+optional pool) on tiles [(i0_src, y0)].
                glob_dst: None -> xn indexed like xk; else offset added to
                i0 for the destination (pool target is a persistent buf)."""
                pool_after = L in pools_after
                nfree = ti * rows * h
                for g0 in range(0, len(tiles), gsz):
                    grp = tiles[g0:g0 + gsz]
                    pss = []
                    for _pi in range(len(grp)):
                        ps_g = cps.tile([128, nfree], f32, tag="cps")
                        pss.append(ps_g)
                    for t, (dy, dx) in enumerate(TAPS):
                        for (i0, y0), ps in zip(grp, pss):
                            rhs = xk[:, i0:i0 + ti,
                                     1 + y0 + dy:1 + y0 + dy + rows,
                                     1 + dx:1 + dx + h]
                            nc.tensor.matmul(
                                ps[:], lt16[:, L - 1, t, :], rhs,
                                start=(t == 0), stop=(t == 8))
                    for pi, ((i0, y0), ps) in enumerate(zip(grp, pss)):
                        psv = ps[:].rearrange(
                            "p (i h w) -> p i h w", i=ti, h=rows)
                        di = i0 if glob_dst is None else i0 + glob_dst
                        if not pool_after:
                            dst = xn[:, di:di + ti, 1 + y0:1 + y0 + rows,
                                     1:1 + h]
                            if pi % 2 == 0:
                                nc.scalar.activation(
                                    dst, psv, Relu, bias=bias[:, L:L + 1])
                            else:
                                # relu(x + b) on DVE: (x add b) max 0
                                nc.vector.tensor_scalar(
                                    dst, psv, bias[:, L:L + 1], 0.0,
                                    mybir.AluOpType.add, mybir.AluOpType.max)
                            continue
                        tc_t = tmp.tile([128, ti, rows, h], f32r,
                                        tag=f"tmp{h}")
                        if pi % 2 == 0:
                            nc.scalar.activation(
                                tc_t[:], psv, Relu, bias=bias[:, L:L + 1])
                        else:
                            nc.vector.tensor_scalar(
                                tc_t[:], psv, bias[:, L:L + 1], 0.0,
                                mybir.AluOpType.add, mybir.AluOpType.max)
                        th = tmp.tile([128, ti, rows, h // 2], f32r,
                                      tag=f"tmph{h}")
                        t4 = tc_t[:].rearrange(
                            "p i h (w two) -> p i h w two", two=2)
                        nc.vector.tensor_max(
                            th[:], t4[:, :, :, :, 0], t4[:, :, :, :, 1])
                        t5 = th[:].rearrange(
                            "p i (h two) w -> p i h two w", two=2)
                        nc.vector.tensor_max(
                            xn[:, di:di + ti, 1 + y0 // 2:1 + (y0 + rows) // 2,
                               1:1 + h // 2],
                            t5[:, :, :, 0, :], t5[:, :, :, 1, :])

            def conv0(ch, x0c):
                """K=108 im2col conv0 for one chunk into x1."""
                ps0 = []
                for _pi in range(8):
                    ps_g = cps.tile([128, 512], f32, tag="cps")
                    ps0.append(ps_g)
                for i in range(CHUNK):
                    for s in range(2):
                        nc.tensor.matmul(
                            ps0[2 * i + s][:], lt0,
                            x0c[:, i, 16 * s:16 * s + 16, :],
                            start=True, stop=True)
                for i in range(CHUNK):
                    for s in range(2):
                        dst0 = x1[:, i, 1 + 16 * s:17 + 16 * s, 1:33]
                        psv0 = ps0[2 * i + s][:].rearrange(
                            "p (h w) -> p h w", h=16)
                        if s == 0:
                            nc.scalar.activation(dst0, psv0, Relu,
                                                 bias=bias[:, 0:1])
                        else:
                            nc.vector.tensor_scalar(
                                dst0, psv0, bias[:, 0:1], 0.0,
                                mybir.AluOpType.add, mybir.AluOpType.max)

            for ch in range(N_CHUNKS):
                cst = CHUNK * ch
                if ch + 1 < N_CHUNKS:
                    # per-image DMAs (4KB per-partition packets: long enough
                    # for queue rate, short enough not to stall PE SBUF
                    # fetch), issued a full chunk ahead of their conv0
                    x0tiles[ch + 1] = x0pool.tile(
                        [108, CHUNK, 32, 32], f32r, tag="x0c", name="x0c")
                    for i_ in range(CHUNK):
                        nc.sync.dma_start(
                            out=x0tiles[ch + 1][:, i_],
                            in_=x0s_d[:, CHUNK * (ch + 1) + i_])
                if ch == 4:
                    # fc weights arrive mid-kernel on the idle Act queue
                    nc.scalar.dma_start(out=lt7[:], in_=lt7_d[:])

                if ch == 0:
                    conv0(0, x0tiles[0])
                xk = x1
                for L in range(1, 5):
                    h = conv_h[L]
                    ti = tile_imgs[L]
                    pool_after = L in pools_after
                    hn = h // 2 if pool_after else h
                    strips = max(1, (h * h * ti) // 512)
                    rows = h // strips
                    tiles = [(it * ti, s * rows) for it in range(CHUNK // ti)
                             for s in range(strips)]
                    if L == 4:
                        xn, glob = x45, cst
                    else:
                        xn = {1: x1b, 2: xm, 3: xm2}[L]
                        glob = None
                    conv_layer(L, xk, xn, tiles, ti, h, rows, glob)
                    xk = xn
                    if L == 2 and ch + 1 < N_CHUNKS:
                        # conv0 for the NEXT chunk: runs on the PE here so
                        # its x1 writes complete before conv1(ch+1)
                        conv0(ch + 1, x0tiles[ch + 1])

            # conv5 / conv6 over all 32 images per group
            for L, xk, xn in ((5, x45, x56), (6, x56, x7)):
                tiles = [(4 * k, 0) for k in range(8)]
                conv_layer(L, xk, xn, tiles, 4, 8, 8, 0 if L == 6 else None,
                           gsz=4)

            # fc7: X7-stationary, K=32 matmuls on the diagonal PE quadrants.
            # ps7[g][32g+i, o] = sum_{c,yx} x7[32g+c, i, yx] * w7[c, o, yx]
            f7i = persist.tile([128, 256], f32r, tag="f7i")
            ps7 = []
            for _pi in range(4):
                ps_g = cps.tile([32, 256], f32, tag="cps")
                ps7.append(ps_g)
            for t, (y, x) in enumerate((y, x) for y in range(4)
                                       for x in range(4)):
                for g in range(4):
                    nc.tensor.matmul(
                        ps7[g][:],
                        x7[32 * g:32 * g + 32, :, 1 + y, 1 + x],
                        lt7[32 * g:32 * g + 32, 4 * y + x, :],
                        start=(t == 0), stop=(t == 15),
                        tile_position=(32 * g, 0))
            # raw z7 (no bias/relu yet) -> f7i; cross-partition write:
            # psum-aligned rows -> sbuf rows 32g+ (Act engine only: DVE
            # cannot remap partitions and silently corrupts)
            for g in range(4):
                nc.scalar.activation(f7i[32 * g:32 * g + 32, :],
                                     ps7[g][:], Copy)

            # transpose -> [o, img]; bias is now per-partition, so fuse
            # relu(z + b7) into the post-transpose copy
            f7t = persist.tile([128, 2, 128], f32r, tag="f7t")
            for hh in range(2):
                pst = cps.tile([128, 128], f32r, tag="cps")
                nc.tensor.transpose(
                    pst[:], f7i[:, 128 * hh:128 * (hh + 1)], idt)
                nc.scalar.activation(f7t[:, hh, :], pst[:], Relu,
                                     bias=b7cols[:, hh:hh + 1])

            outt = persist.tile([10, N_IMG], f32, tag="outt")
            ps8 = cps.tile([10, N_IMG], f32, tag="cps")
            for hh in range(2):
                nc.tensor.matmul(ps8[:], lt8[:, hh, :], f7t[:, hh, :],
                                 start=(hh == 0), stop=(hh == 1))
            nc.scalar.activation(outt[:], ps8[:], Ident,
                                 bias=bias[0:10, 9:10])
            nc.sync.dma_start(out=out_d[:], in_=outt[:])

    nc.finalize()
    return nc


_NC_CACHE = None


def _get_program():
    global _NC_CACHE
    if _NC_CACHE is None:
        _NC_CACHE = _build_program()
    return _NC_CACHE


def _prep_host_inputs(x, ws, bs):
    """Build per-core input maps.  ws/bs: lists of the 9 weight/bias arrays."""
    # conv0 im2col, identical for every core: [108, 32, 32, 32]
    xp = np.zeros((N_IMG, 3, 34, 34), np.float32)
    xp[:, :, 1:33, 1:33] = x
    x0s = np.empty((108, IMG_PER_GROUP, 32, 32), np.float32)
    for g in range(4):
        sl = xp[g * 32:(g + 1) * 32]
        for c in range(3):
            for ky in range(3):
                for kx in range(3):
                    x0s[27 * g + 9 * c + 3 * ky + kx] = \
                        sl[:, c, ky:ky + 32, kx:kx + 32]
    x0s = round_fp32r(x0s)

    in_maps = []
    for m in range(N_MODELS):
        # lt0 (im2col conv0 weights) + per-layer biases in one tensor
        lt0b = np.zeros((128, 138), np.float32)
        w0m = round_fp32r(ws[0][m].transpose(0, 2, 1).reshape(27, 32))
        for g in range(4):
            lt0b[27 * g:27 * g + 27, 32 * g:32 * g + 32] = w0m
        for L in range(7):
            bL = bs[L][m][:, 0]  # [32]
            for g in range(4):
                lt0b[32 * g:32 * g + 32, 128 + L] = bL
        lt0b[0:10, 128 + 9] = bs[8][m][:, 0]

        lt16 = np.zeros((128, 6, 9, 128), np.float32)
        for L in range(1, 7):
            wm = ws[L][m].transpose(0, 2, 1)  # [32c, 9t, 32o]
            for g in range(4):
                lt16[32 * g:32 * g + 32, L - 1, :, 32 * g:32 * g + 32] = wm

        # lt7[32g+c, yx, o] = w7[m, c, o, yx]  (same block for every g)
        lt7 = np.empty((128, 16, 256), np.float32)
        blk7 = ws[7][m].transpose(0, 2, 1)  # [32c, 16yx, 256o]
        for g in range(4):
            lt7[32 * g:32 * g + 32] = blk7

        # identity (exact in fp32r) + fc7 bias halves + fc8 weights
        smalls = np.zeros((128, 150), np.float32)
        smalls[:, 0:128] = np.eye(128, dtype=np.float32)
        for hh in range(2):
            smalls[:, 128 + hh] = bs[7][m][128 * hh:128 * (hh + 1), 0]
            smalls[:, 130 + 10 * hh:140 + 10 * hh] = round_fp32r(
                ws[8][m][128 * hh:128 * (hh + 1), :, 0])

        in_maps.append({
            "x0s": x0s,
            "lt0b": lt0b,
            "lt16": round_fp32r(lt16),
            "lt7": round_fp32r(lt7),
            "smalls": smalls,
        })
    return in_maps


def kernel(x, w0, w1, w2, w3, w4, w5, w6, w7, w8,
           b0, b1, b2, b3, b4, b5, b6, b7, b8):
    from concourse.bass_utils import run_bass_kernel_spmd

    ws = [np.asarray(w, np.float32) for w in
          (w0, w1, w2, w3, w4, w5, w6, w7, w8)]
    bs = [np.asarray(b, np.float32) for b in
          (b0, b1, b2, b3, b4, b5, b6, b7, b8)]
    nc = _get_program()
    in_maps = _prep_host_inputs(np.asarray(x, np.float32), ws, bs)
    res = run_bass_kernel_spmd(nc, in_maps, list(range(N_MODELS)))
    out = np.stack([res.results[m]["out"].T for m in range(N_MODELS)])
    return np.ascontiguousarray(out, dtype=np.float32)


# revision 23
# speedup vs baseline: 1.0016x; 1.0016x over previous
"""Trainium2 Bass kernel for the 8-model batch-functional CNN.

Sharding: one hypernetwork model per NeuronCore (8 models / 8 cores).
Each core runs the full 7-conv + 2-fc stack for its model over all 128
images, activations resident in SBUF.

Layout: SBUF activations are [128 partitions = 4 image-groups x 32
channels, imgs, H+2, W+2] (zero-padded borders).  3x3 convs are 9
PSUM-accumulated fp32r matmuls with block-diagonal [128,128] weights
and dy/dx-shifted rhs views; matmuls are grouped taps-outer over up to
8 PSUM banks so walrus's weight-tile cache (ldw-opt) elides redundant
LDWEIGHTS.  conv0 uses a host-side im2col (K=108).  MaxPool2d(2) is
two strided tensor_max ops on DVE.  fc7 runs X7-stationary on the
diagonal 32x32 PE quadrants, fc8 after a PE transpose (the fc7 bias is
applied post-transpose so it becomes a per-partition Act bias).

Startup is DMA-latency critical: chunk-0 images load per-image (4KB
per-partition packets) spread over the Pool/SP/Act queues with the
conv1 weights on their own queue; steady chunks load per-image a full
chunk ahead.  DMA transfers with >4KB per-partition contiguous runs
(16KB packets) and all-F=512 matmul mixes both trip a ~20% whole-core
clock derate at NEFF load, so chunk loads stay at 4KB packets and
conv3-6 keep F=256 tiles.  conv0 for chunk ch+1 runs between conv2(ch)
and conv3(ch) so its x1 writes land before the PE reaches conv1(ch+1).
"""
import sys

sys.path.insert(0, "/opt/trn_rl_repo")
import numpy as np

N_MODELS = 8
N_IMG = 128
IMG_PER_GROUP = 32   # images assigned to each of the 4 partition groups
CHUNK = 4            # images per group per chunk through conv0..conv4
N_CHUNKS = IMG_PER_GROUP // CHUNK


def round_fp32r(a):
    a = np.ascontiguousarray(a, dtype=np.float32)
    b = a.view(np.uint32)
    low = b & np.uint32(0xFFF)
    bit12 = (b >> np.uint32(12)) & np.uint32(1)
    up = (low > 0x800) | ((low == 0x800) & (bit12 == 1))
    out = (b & np.uint32(0xFFFFF000)) + (up.astype(np.uint32) << np.uint32(12))
    return out.view(np.float32)


def _enable_ldw_opt():
    """Turn on walrus's weight-tile cache so back-to-back matmuls with the
    same stationary operand skip the redundant LDWEIGHTS."""
    import concourse.bass_utils as bu
    if getattr(bu.run_command, "_ldw_patched", False):
        return
    orig = bu.run_command

    def run_command(cmd, *a, **kw):
        cmd = ["--enable-ldw-opt=true" if c == "--enable-ldw-opt=false" else c
               for c in cmd]
        return orig(cmd, *a, **kw)

    run_command._ldw_patched = True
    bu.run_command = run_command


def _build_program():
    import concourse.bacc as bacc
    import concourse.tile as tile
    from concourse import mybir

    _enable_ldw_opt()

    f32 = mybir.dt.float32
    f32r = mybir.dt.float32r
    Relu = mybir.ActivationFunctionType.Relu
    Ident = mybir.ActivationFunctionType.Identity
    Copy = mybir.ActivationFunctionType.Copy

    nc = bacc.Bacc("TRN2", target_bir_lowering=False, debug=False)

    x0s_d = nc.declare_dram_parameter("x0s", [108, IMG_PER_GROUP, 32, 32], f32r, isOutput=False)
    lt0b_d = nc.declare_dram_parameter("lt0b", [128, 138], f32r, isOutput=False)
    lt16_d = nc.declare_dram_parameter("lt16", [128, 6, 9, 128], f32r, isOutput=False)
    lt7_d = nc.declare_dram_parameter("lt7", [128, 16, 256], f32r, isOutput=False)
    smalls_d = nc.declare_dram_parameter("smalls", [128, 150], f32r, isOutput=False)
    out_d = nc.declare_dram_parameter("out", [10, N_IMG], f32, isOutput=True)

    conv_h = {1: 32, 2: 32, 3: 16, 4: 16, 5: 8, 6: 8}
    pools_after = {2, 4, 6}
    tile_imgs = {1: 1, 2: 1, 3: 1, 4: 1, 5: 4, 6: 4}
    TAPS = [(dy, dx) for dy in (-1, 0, 1) for dx in (-1, 0, 1)]

    with tile.TileContext(nc) as tc:
        with tc.tile_pool(name="wpool", bufs=1) as wpool, \
             tc.tile_pool(name="acts", bufs=1) as acts, \
             tc.tile_pool(name="x0pool", bufs=2) as x0pool, \
             tc.tile_pool(name="tmp", bufs=2) as tmp, \
             tc.tile_pool(name="persist", bufs=1) as persist, \
             tc.tile_pool(name="cps", bufs=8, space="PSUM") as cps:

            lt0b = wpool.tile([128, 138], f32r, tag="lt0b")
            lt16 = wpool.tile([128, 6, 9, 128], f32r, tag="lt16")
            lt7 = wpool.tile([128, 16, 256], f32r, tag="lt7")
            smalls = wpool.tile([128, 150], f32r, tag="smalls")

            lt0 = lt0b[0:108, 0:128]
            bias = lt0b[:, 128:138].bitcast(f32)     # [128, 10] conv/fc biases
            idt = smalls[:, 0:128]                   # f32r identity
            b7cols = smalls[:, 128:130].bitcast(f32)  # [128, 2] fc7 bias halves
            lt8 = smalls[:, 130:150].rearrange("p (h o) -> p h o", h=2)

            # persistent buffers (across chunks)
            x45 = persist.tile([128, IMG_PER_GROUP, 10, 10], f32r, tag="x45")
            x56 = persist.tile([128, IMG_PER_GROUP, 10, 10], f32r, tag="x56")
            x7 = persist.tile([128, IMG_PER_GROUP, 6, 6], f32r, tag="x7")

            # conv0..conv4 SBUF activation buffers (borders zeroed once,
            # interiors fully rewritten every chunk)
            x1 = acts.tile([128, CHUNK, 34, 34], f32r, tag="big")
            x1b = acts.tile([128, CHUNK, 34, 34], f32r, tag="big2")
            xm = acts.tile([128, CHUNK, 18, 18], f32r, tag="med")
            xm2 = acts.tile([128, CHUNK, 18, 18], f32r, tag="med2")

            # ---- startup DMAs, ordered per queue for earliest PE start ----
            x0tiles = [None] * N_CHUNKS
            x0tiles[0] = x0pool.tile([108, CHUNK, 32, 32], f32r, tag="x0c",
                                     name="x0c")
            x0c0 = x0tiles[0]
            # Pool queue: conv0 weights+biases (small, lands first), then
            # conv1 weights (needed by the first conv1 half-pass)
            nc.gpsimd.dma_start(out=lt0b[:], in_=lt0b_d[:])
            nc.gpsimd.dma_start(out=lt16[:, 0], in_=lt16_d[:, 0])
            nc.gpsimd.dma_start(out=lt16[:, 2], in_=lt16_d[:, 2])
            nc.gpsimd.dma_start(out=lt16[:, 4], in_=lt16_d[:, 4])
            # SP queue: img0 first (gates the first matmul), img3 (only
            # needed by the second conv0 half-pass), small tensors
            nc.sync.dma_start(out=x0c0[:, 0], in_=x0s_d[:, 0])
            nc.sync.dma_start(out=x0c0[:, 3], in_=x0s_d[:, 3])
            nc.sync.dma_start(out=smalls[:], in_=smalls_d[:])
            nc.sync.dma_start(out=lt16[:, 5], in_=lt16_d[:, 5])
            # Act queue (ACT_TABLE_LOAD runs first): img1, img2, then
            # conv2/conv4 weights (needed only after the conv1 half-passes)
            nc.scalar.dma_start(out=x0c0[:, 1], in_=x0s_d[:, 1])
            nc.scalar.dma_start(out=x0c0[:, 2], in_=x0s_d[:, 2])
            nc.scalar.dma_start(out=lt16[:, 1], in_=lt16_d[:, 1])
            nc.scalar.dma_start(out=lt16[:, 3], in_=lt16_d[:, 3])

            # ---- one-time border zeroing (alternate DVE / Pool) ----
            k_ms = 0

            def memset0(ap):
                nonlocal k_ms
                eng = nc.vector if k_ms % 2 == 0 else nc.gpsimd
                eng.memset(ap.bitcast(f32), 0.0)
                k_ms += 1

            for buf, n in ((x1, 34), (x1b, 34), (xm, 18), (xm2, 18)):
                memset0(buf[:, :, 0, :])
                memset0(buf[:, :, n - 1, :])
                memset0(buf[:, :, 1:n - 1, 0])
                memset0(buf[:, :, 1:n - 1, n - 1])
            for buf, n in ((x45, 10), (x56, 10), (x7, 6)):
                memset0(buf[:, :, 0, :])
                memset0(buf[:, :, n - 1, :])
                memset0(buf[:, :, 1:n - 1, 0])
                memset0(buf[:, :, 1:n - 1, n - 1])

            def conv_layer(L, xk, xn, tiles, ti, h, rows, glob_dst,
                           gsz=8):
                """One 3x3 conv (+optional pool) on tiles [(i0_src, y0)].
                glob_dst: None -> xn indexed like xk; else offset added to
                i0 for the destination (pool target is a persistent buf)."""
                pool_after = L in pools_after
                nfree = ti * rows * h
                for g0 in range(0, len(tiles), gsz):
                    grp = tiles[g0:g0 + gsz]
                    pss = []
                    for _pi in range(len(grp)):
                        ps_g = cps.tile([128, nfree], f32, tag="cps")
                        pss.append(ps_g)
                    for t, (dy, dx) in enumerate(TAPS):
                        for (i0, y0), ps in zip(grp, pss):
                            rhs = xk[:, i0:i0 + ti,
                                     1 + y0 + dy:1 + y0 + dy + rows,
                                     1 + dx:1 + dx + h]
                            nc.tensor.matmul(
                                ps[:], lt16[:, L - 1, t, :], rhs,
                                start=(t == 0), stop=(t == 8))
                    for pi, ((i0, y0), ps) in enumerate(zip(grp, pss)):
                        psv = ps[:].rearrange(
                            "p (i h w) -> p i h w", i=ti, h=rows)
                        di = i0 if glob_dst is None else i0 + glob_dst
                        if not pool_after:
                            dst = xn[:, di:di + ti, 1 + y0:1 + y0 + rows,
                                     1:1 + h]
                            if pi % 2 == 0:
                                nc.scalar.activation(
                                    dst, psv, Relu, bias=bias[:, L:L + 1])
                            else:
                                # relu(x + b) on DVE: (x add b) max 0
                                nc.vector.tensor_scalar(
                                    dst, psv, bias[:, L:L + 1], 0.0,
                                    mybir.AluOpType.add, mybir.AluOpType.max)
                            continue
                        tc_t = tmp.tile([128, ti, rows, h], f32r,
                                        tag=f"tmp{h}")
                        if pi % 2 == 0:
                            nc.scalar.activation(
                                tc_t[:], psv, Relu, bias=bias[:, L:L + 1])
                        else:
                            nc.vector.tensor_scalar(
                                tc_t[:], psv, bias[:, L:L + 1], 0.0,
                                mybir.AluOpType.add, mybir.AluOpType.max)
                        th = tmp.tile([128, ti, rows, h // 2], f32r,
                                      tag=f"tmph{h}")
                        t4 = tc_t[:].rearrange(
                            "p i h (w two) -> p i h w two", two=2)
                        nc.vector.tensor_max(
                            th[:], t4[:, :, :, :, 0], t4[:, :, :, :, 1])
                        t5 = th[:].rearrange(
                            "p i (h two) w -> p i h two w", two=2)
                        nc.vector.tensor_max(
                            xn[:, di:di + ti, 1 + y0 // 2:1 + (y0 + rows) // 2,
                               1:1 + h // 2],
                            t5[:, :, :, 0, :], t5[:, :, :, 1, :])

            def conv0(ch, x0c, imgs=range(CHUNK)):
                """K=108 im2col conv0 for one chunk into x1."""
                ps0 = {}
                for i in imgs:
                    for s in range(2):
                        ps_g = cps.tile([128, 512], f32, tag="cps",
                                        name="cps0")
                        ps0[2 * i + s] = ps_g
                for i in imgs:
                    for s in range(2):
                        nc.tensor.matmul(
                            ps0[2 * i + s][:], lt0,
                            x0c[:, i, 16 * s:16 * s + 16, :],
                            start=True, stop=True)
                for i in imgs:
                    for s in range(2):
                        dst0 = x1[:, i, 1 + 16 * s:17 + 16 * s, 1:33]
                        psv0 = ps0[2 * i + s][:].rearrange(
                            "p (h w) -> p h w", h=16)
                        if s == 0:
                            nc.scalar.activation(dst0, psv0, Relu,
                                                 bias=bias[:, 0:1])
                        else:
                            nc.vector.tensor_scalar(
                                dst0, psv0, bias[:, 0:1], 0.0,
                                mybir.AluOpType.add, mybir.AluOpType.max)

            for ch in range(N_CHUNKS):
                cst = CHUNK * ch
                if ch + 1 < N_CHUNKS:
                    # per-image DMAs (4KB per-partition packets: long enough
                    # for queue rate, short enough not to stall PE SBUF
                    # fetch), issued a full chunk ahead of their conv0
                    x0tiles[ch + 1] = x0pool.tile(
                        [108, CHUNK, 32, 32], f32r, tag="x0c", name="x0c")
                    for i_ in range(CHUNK):
                        nc.sync.dma_start(
                            out=x0tiles[ch + 1][:, i_],
                            in_=x0s_d[:, CHUNK * (ch + 1) + i_])
                if ch == 4:
                    # fc weights arrive mid-kernel on the idle Act queue
                    nc.scalar.dma_start(out=lt7[:], in_=lt7_d[:])

                if ch == 0:
                    # conv0/conv1 interleaved half-passes: conv1 on images
                    # 0-1 starts as soon as they land, while images 2-3 are
                    # still in flight on the Act queue
                    conv0(0, x0tiles[0], imgs=(0, 1))
                xk = x1
                for L in range(1, 5):
                    h = conv_h[L]
                    ti = tile_imgs[L]
                    pool_after = L in pools_after
                    hn = h // 2 if pool_after else h
                    strips = max(1, (h * h * ti) // 512)
                    rows = h // strips
                    tiles = [(it * ti, s * rows) for it in range(CHUNK // ti)
                             for s in range(strips)]
                    if L == 4:
                        xn, glob = x45, cst
                    else:
                        xn = {1: x1b, 2: xm, 3: xm2}[L]
                        glob = None
                    if ch == 0 and L == 1:
                        conv_layer(L, xk, xn, tiles[:4], ti, h, rows, glob)
                        conv0(0, x0tiles[0], imgs=(2, 3))
                        conv_layer(L, xk, xn, tiles[4:], ti, h, rows, glob)
                    else:
                        conv_layer(L, xk, xn, tiles, ti, h, rows, glob)
                    xk = xn
                    if L == 2 and ch + 1 < N_CHUNKS:
                        # conv0 for the NEXT chunk: runs on the PE here so
                        # its x1 writes complete before conv1(ch+1)
                        conv0(ch + 1, x0tiles[ch + 1])

            # conv5 / conv6 over all 32 images per group
            for L, xk, xn in ((5, x45, x56), (6, x56, x7)):
                tiles = [(4 * k, 0) for k in range(8)]
                conv_layer(L, xk, xn, tiles, 4, 8, 8, 0 if L == 6 else None,
                           gsz=4)

            # fc7: X7-stationary, K=32 matmuls on the diagonal PE quadrants.
            # ps7[g][32g+i, o] = sum_{c,yx} x7[32g+c, i, yx] * w7[c, o, yx]
            f7i = persist.tile([128, 256], f32r, tag="f7i")
            ps7 = []
            for _pi in range(4):
                ps_g = cps.tile([32, 256], f32, tag="cps")
                ps7.append(ps_g)
            for t, (y, x) in enumerate((y, x) for y in range(4)
                                       for x in range(4)):
                for g in range(4):
                    nc.tensor.matmul(
                        ps7[g][:],
                        x7[32 * g:32 * g + 32, :, 1 + y, 1 + x],
                        lt7[32 * g:32 * g + 32, 4 * y + x, :],
                        start=(t == 0), stop=(t == 15),
                        tile_position=(32 * g, 0))
            # raw z7 (no bias/relu yet) -> f7i; cross-partition write:
            # psum-aligned rows -> sbuf rows 32g+ (Act engine only)
            for g in range(4):
                nc.scalar.activation(f7i[32 * g:32 * g + 32, :],
                                     ps7[g][:], Copy)

            # transpose -> [o, img]; bias is now per-partition, so fuse
            # relu(z + b7) into the post-transpose copy
            f7t = persist.tile([128, 2, 128], f32r, tag="f7t")
            for hh in range(2):
                pst = cps.tile([128, 128], f32r, tag="cps")
                nc.tensor.transpose(
                    pst[:], f7i[:, 128 * hh:128 * (hh + 1)], idt)
                nc.scalar.activation(f7t[:, hh, :], pst[:], Relu,
                                     bias=b7cols[:, hh:hh + 1])

            outt = persist.tile([10, N_IMG], f32, tag="outt")
            ps8 = cps.tile([10, N_IMG], f32, tag="cps")
            for hh in range(2):
                nc.tensor.matmul(ps8[:], lt8[:, hh, :], f7t[:, hh, :],
                                 start=(hh == 0), stop=(hh == 1))
            nc.scalar.activation(outt[:], ps8[:], Ident,
                                 bias=bias[0:10, 9:10])
            nc.sync.dma_start(out=out_d[:], in_=outt[:])

    nc.finalize()
    return nc


_NC_CACHE = None


def _get_program():
    global _NC_CACHE
    if _NC_CACHE is None:
        _NC_CACHE = _build_program()
    return _NC_CACHE


def _prep_host_inputs(x, ws, bs):
    """Build per-core input maps.  ws/bs: lists of the 9 weight/bias arrays."""
    # conv0 im2col, identical for every core: [108, 32, 32, 32]
    xp = np.zeros((N_IMG, 3, 34, 34), np.float32)
    xp[:, :, 1:33, 1:33] = x
    x0s = np.empty((108, IMG_PER_GROUP, 32, 32), np.float32)
    for g in range(4):
        sl = xp[g * 32:(g + 1) * 32]
        for c in range(3):
            for ky in range(3):
                for kx in range(3):
                    x0s[27 * g + 9 * c + 3 * ky + kx] = \
                        sl[:, c, ky:ky + 32, kx:kx + 32]
    x0s = round_fp32r(x0s)

    in_maps = []
    for m in range(N_MODELS):
        # lt0 (im2col conv0 weights) + per-layer biases in one tensor
        lt0b = np.zeros((128, 138), np.float32)
        w0m = round_fp32r(ws[0][m].transpose(0, 2, 1).reshape(27, 32))
        for g in range(4):
            lt0b[27 * g:27 * g + 27, 32 * g:32 * g + 32] = w0m
        for L in range(7):
            bL = bs[L][m][:, 0]  # [32]
            for g in range(4):
                lt0b[32 * g:32 * g + 32, 128 + L] = bL
        lt0b[0:10, 128 + 9] = bs[8][m][:, 0]

        lt16 = np.zeros((128, 6, 9, 128), np.float32)
        for L in range(1, 7):
            wm = ws[L][m].transpose(0, 2, 1)  # [32c, 9t, 32o]
            for g in range(4):
                lt16[32 * g:32 * g + 32, L - 1, :, 32 * g:32 * g + 32] = wm

        # lt7[32g+c, yx, o] = w7[m, c, o, yx]  (same block for every g)
        lt7 = np.empty((128, 16, 256), np.float32)
        blk7 = ws[7][m].transpose(0, 2, 1)  # [32c, 16yx, 256o]
        for g in range(4):
            lt7[32 * g:32 * g + 32] = blk7

        # identity (exact in fp32r) + fc7 bias halves + fc8 weights
        smalls = np.zeros((128, 150), np.float32)
        smalls[:, 0:128] = np.eye(128, dtype=np.float32)
        for hh in range(2):
            smalls[:, 128 + hh] = bs[7][m][128 * hh:128 * (hh + 1), 0]
            smalls[:, 130 + 10 * hh:140 + 10 * hh] = round_fp32r(
                ws[8][m][128 * hh:128 * (hh + 1), :, 0])

        in_maps.append({
            "x0s": x0s,
            "lt0b": lt0b,
            "lt16": round_fp32r(lt16),
            "lt7": round_fp32r(lt7),
            "smalls": smalls,
        })
    return in_maps


def kernel(x, w0, w1, w2, w3, w4, w5, w6, w7, w8,
           b0, b1, b2, b3, b4, b5, b6, b7, b8):
    from concourse.bass_utils import run_bass_kernel_spmd

    ws = [np.asarray(w, np.float32) for w in
          (w0, w1, w2, w3, w4, w5, w6, w7, w8)]
    bs = [np.asarray(b, np.float32) for b in
          (b0, b1, b2, b3, b4, b5, b6, b7, b8)]
    nc = _get_program()
    in_maps = _prep_host_inputs(np.asarray(x, np.float32), ws, bs)
    res = run_bass_kernel_spmd(nc, in_maps, list(range(N_MODELS)))
    out = np.stack([res.results[m]["out"].T for m in range(N_MODELS)])
    return np.ascontiguousarray(out, dtype=np.float32)
